# revision 7
# baseline (speedup 1.0000x reference)
"""Causal self-attention with RoPE on 8 Trainium2 NeuronCores.

Problem (hardcoded): x [2, 2048, 1024] f32, w_qkv [1024, 3072], w_out [1024, 1024],
16 heads x head_dim 64, RoPE base 10000, causal softmax, out = attn @ w_out.

Sharding: DP over batch (2) x TP over head-groups (4 heads/core) = 8 cores.
Each core computes QKV for its 4 heads, full causal attention, and a partial
output projection against its 256 rows of w_out. Host sums the 4 partials per
batch element.

Per-core device layout (all matmul operands fp32r = full-rate PE):
  xT   [1024, 2048]  x[b].T
  w_qk [1024, 512]   columns: [Q h0..h3 (4*64) | K h0..h3]
  w_v  [1024, 256]
  w_o  [256, 1024]   rows of w_out for this head group
  QKT  4 tiles [128, 2048] = Q^T/K^T in [channel, seq] layout (2 heads/tile)
  Vsb  16 tiles [128, 4, 65] = V[k-tile] per head + ones column (denominator)
  scores^T strips [k=128, q<=1024] in PSUM -> exp on ACT -> P^T (fp32r)
  out^T[qc] [65, 512] PSUM accumulators: rows 0..63 = head out, row 64 = denom
"""
import numpy as np

import concourse.bacc as bacc
import concourse.tile as tile
from concourse import mybir
from concourse.bass_utils import run_bass_kernel_spmd

F32 = mybir.dt.float32
F32R = mybir.dt.float32r
EXP = mybir.ActivationFunctionType.Exp

B, S, D = 2, 2048, 1024
H, HD = 16, 64
HPC = 4          # heads per core
CQK = 2 * HPC * HD   # 512 qk channels per core
CV = HPC * HD        # 256 v channels per core
NKT = S // 128       # 16 k-tiles
NSC = S // 512       # 4 seq chunks
SCALE = 1.0 / np.sqrt(HD)
ROPE_BASE = 10000.0


def _build_nc():
    nc = bacc.Bacc(None, target_bir_lowering=False, debug=False)

    xT = nc.declare_dram_parameter("xT", [D, S], F32, isOutput=False)
    w_qk = nc.declare_dram_parameter("w_qk", [D, CQK], F32, isOutput=False)
    w_v = nc.declare_dram_parameter("w_v", [D, CV], F32, isOutput=False)
    w_o = nc.declare_dram_parameter("w_o", [CV, D], F32, isOutput=False)
    cos2 = nc.declare_dram_parameter("cos2", [128, S], F32, isOutput=False)
    sin2n = nc.declare_dram_parameter("sin2n", [128, S], F32, isOutput=False)
    umask = nc.declare_dram_parameter("umask", [128, 896], F32, isOutput=False)
    ones4 = nc.declare_dram_parameter("ones4", [128, 4, 1], F32, isOutput=False)
    out = nc.declare_dram_parameter("out", [S, D], F32, isOutput=True)

    with tile.TileContext(nc) as tc:
        with (
            tc.tile_pool(name="const", bufs=1) as const,
            tc.tile_pool(name="qkt", bufs=1) as qkt_pool,
            tc.tile_pool(name="vsb", bufs=1) as vsb_pool,
            tc.tile_pool(name="rot", bufs=2) as rot_pool,
            tc.tile_pool(name="ps", bufs=1, space="PSUM") as ps,
        ):
            # ---- constants ----
            cos_sb = const.tile([128, S], F32R)
            sin_sb = const.tile([128, S], F32R)
            u_sb = const.tile([128, 896], F32R)
            nc.sync.dma_start(out=cos_sb, in_=cos2[:, :].bitcast(F32R))
            nc.sync.dma_start(out=sin_sb, in_=sin2n[:, :].bitcast(F32R))
            nc.sync.dma_start(out=u_sb, in_=umask[:, :].bitcast(F32R))
            ones_sb = const.tile([128, 4, 1], F32R)
            nc.sync.dma_start(out=ones_sb, in_=ones4[:, :, :].bitcast(F32R))
            wo_sb = [const.tile([128, D], F32R, name=f"wo{i}") for i in range(2)]
            for i in range(2):
                nc.sync.dma_start(
                    out=wo_sb[i], in_=w_o[i * 128 : (i + 1) * 128, :].bitcast(F32R)
                )

            # persistent outputs of phase 1
            QKT = [qkt_pool.tile([128, S], F32R, name=f"qkt{t}") for t in range(4)]
            Vsb = [vsb_pool.tile([128, HPC, 65], F32R, name=f"v{k}") for k in range(NKT)]

            # ---- phase 1: QKV projection ----
            with tc.tile_pool(name="p1", bufs=1) as p1:
                wqk_sb = [p1.tile([128, CQK], F32R, name=f"wqk{d}") for d in range(8)]
                wv_sb = [p1.tile([128, CV], F32R, name=f"wv{d}") for d in range(8)]
                for d in range(8):
                    nc.sync.dma_start(
                        out=wqk_sb[d],
                        in_=w_qk[d * 128 : (d + 1) * 128, :].bitcast(F32R),
                    )
                # xT tiles chunked [d-tile 128, s-chunk 512]; sc0 first so PE starts ASAP
                xt_sb = {}
                for sc in range(NSC):
                    for d in range(8):
                        t = p1.tile([128, 512], F32R, name=f"xt{sc}_{d}")
                        nc.sync.dma_start(
                            out=t,
                            in_=xT[
                                d * 128 : (d + 1) * 128, sc * 512 : (sc + 1) * 512
                            ].bitcast(F32R),
                        )
                        xt_sb[(sc, d)] = t
                    if sc == 0:
                        for d in range(8):
                            nc.sync.dma_start(
                                out=wv_sb[d],
                                in_=w_v[d * 128 : (d + 1) * 128, :].bitcast(F32R),
                            )

                def rope(t):
                    rot = rot_pool.tile([128, S], F32R, name="rope_rot")
                    for blk in range(4):
                        sp = (blk ^ 1) * 32  # 32<->0, 96<->64 swap per head
                        nc.gpsimd.dma_start(
                            out=rot[blk * 32 : blk * 32 + 32, :],
                            in_=QKT[t][sp : sp + 32, :],
                        )
                    nc.vector.tensor_mul(rot, rot, sin_sb)
                    nc.vector.tensor_mul(QKT[t], QKT[t], cos_sb)
                    nc.vector.tensor_add(QKT[t], QKT[t], rot)

                def qk_group(ct, sc):
                    qp = ps.tile([128, 1024], F32, tag="strip", bufs=2, name=f"qk{sc}_{ct}")
                    for d in range(8):
                        nc.tensor.matmul(
                            qp[:, 0:512],
                            wqk_sb[d][:, ct * 128 : (ct + 1) * 128],
                            xt_sb[(sc, d)],
                            start=(d == 0),
                            stop=(d == 7),
                        )
                    nc.vector.tensor_copy(
                        QKT[ct][:, sc * 512 : (sc + 1) * 512], qp[:, 0:512]
                    )

                def v_group(st):
                    sc, sti = st // 4, st % 4
                    vp = ps.tile([128, 4, 64], F32, tag="strip", bufs=2, name=f"vps{st}")
                    for d in range(8):
                        nc.tensor.matmul(
                            vp,
                            xt_sb[(sc, d)][:, sti * 128 : (sti + 1) * 128],
                            wv_sb[d],
                            start=(d == 0),
                            stop=(d == 7),
                        )
                    nc.vector.tensor_copy(Vsb[st][:, :, 0:64], vp)
                    nc.vector.tensor_copy(Vsb[st][:, :, 64:65], ones_sb)

                # Q/K for heads 0,1 first, each tile roped as soon as complete;
                # V interleaved so attention heads 0/1 can start while Q/K for
                # heads 2/3 (ct 1,3) still projects.
                for ct in (0, 2):
                    for sc in range(NSC):
                        qk_group(ct, sc)
                    rope(ct)
                for st in range(NKT):
                    v_group(st)
                for ct in (1, 3):
                    for sc in range(NSC):
                        qk_group(ct, sc)
                    rope(ct)

            # ---- phase 2: attention per head ----
            with (
                tc.tile_pool(name="pp", bufs=3) as pp,
                tc.tile_pool(name="attn", bufs=1) as attn_pool,
                tc.tile_pool(name="nrm", bufs=2) as nrm,
                tc.tile_pool(name="outp", bufs=3) as outp,
            ):
                attnT = [
                    [
                        attn_pool.tile([128, 512], F32R, name=f"attnT{qc}_{ct}")
                        for ct in range(2)
                    ]
                    for qc in range(NSC)
                ]
                for h in range(HPC):
                    qt = QKT[h // 2]
                    kt_t = QKT[2 + h // 2]
                    hh = h % 2
                    outT = [
                        ps.tile([128, 512], F32, tag="outT", bufs=4, name=f"outT{h}_{qc}")
                        for qc in range(NSC)
                    ]
                    for kt in range(NKT):
                        qc0 = kt // 4
                        c0 = qc0 * 512
                        width = S - c0
                        off = c0
                        first = True
                        while off < S:
                            w = min(1024, S - off)
                            sps = ps.tile(
                                [128, 1024], F32, tag="strip", bufs=2, name=f"s{h}_{kt}_{off}"
                            )
                            for j in range(w // 512):
                                nc.tensor.matmul(
                                    sps[:, j * 512 : (j + 1) * 512],
                                    kt_t[hh * 64 : hh * 64 + 64, kt * 128 : (kt + 1) * 128],
                                    qt[hh * 64 : hh * 64 + 64, off + j * 512 : off + (j + 1) * 512],
                                    start=True,
                                    stop=True,
                                )
                            p_t = pp.tile([128, 1024], F32R, name="p_t")
                            nc.scalar.activation(
                                p_t[:, 0:w], sps[:, 0:w], EXP, scale=SCALE
                            )
                            if first:
                                r = kt % 4
                                nc.vector.tensor_mul(
                                    p_t[:, 0:512],
                                    p_t[:, 0:512],
                                    u_sb[:, 384 - 128 * r : 896 - 128 * r],
                                )
                                first = False
                            for j in range(w // 512):
                                qc = (off + j * 512) // 512
                                nc.tensor.matmul(
                                    outT[qc][0:65, :],
                                    Vsb[kt][:, h, :],
                                    p_t[:, j * 512 : (j + 1) * 512],
                                    start=(kt == 0),
                                    stop=(kt == qc * 4 + 3),
                                )
                            off += w
                        # normalize finished q-chunk (kt = qc*4+3 just stopped)
                        if kt % 4 == 3:
                            qc = qc0
                            dn = nrm.tile([1, 512], F32, name="dn")
                            nc.vector.tensor_copy(dn, outT[qc][64:65, :])
                            bc = nrm.tile([64, 512], F32, name="bc")
                            nc.gpsimd.partition_broadcast(bc, dn)
                            rc = nrm.tile([64, 512], F32, name="rc")
                            nc.vector.reciprocal(rc, bc)
                            nc.vector.tensor_mul(
                                attnT[qc][h // 2][hh * 64 : hh * 64 + 64, :],
                                outT[qc][0:64, :],
                                rc,
                            )

                # ---- phase 3: output projection ----
                for st in range(16):
                    qc = st // 4
                    sl = (st % 4) * 128
                    for ec in range(2):
                        op = ps.tile(
                            [128, 1024], F32, tag="strip", bufs=2, name=f"op{st}_{ec}"
                        )
                        for ct in range(2):
                            nc.tensor.matmul(
                                op[:, 0:512],
                                attnT[qc][ct][:, sl : sl + 128],
                                wo_sb[ct][:, ec * 512 : (ec + 1) * 512],
                                start=(ct == 0),
                                stop=(ct == 1),
                            )
                        ob = outp.tile([128, 512], F32, name="ob")
                        nc.vector.tensor_copy(ob, op[:, 0:512])
                        nc.sync.dma_start(
                            out=out[st * 128 : (st + 1) * 128, ec * 512 : (ec + 1) * 512],
                            in_=ob,
                        )
    nc.compile()
    return nc


def _host_tables():
    half = HD // 2
    inv_freq = 1.0 / (ROPE_BASE ** (np.arange(0, half, dtype=np.float64) / half))
    ang = np.arange(S, dtype=np.float64)[:, None] * inv_freq[None, :]  # [S, 32]
    cosT = np.cos(ang).T.astype(np.float32)  # [32, S]
    sinT = np.sin(ang).T.astype(np.float32)
    cos64 = np.concatenate([cosT, cosT], axis=0)  # [64, S]
    sin64s = np.concatenate([-sinT, sinT], axis=0)  # sign-folded rotate_half
    cos2 = np.ascontiguousarray(np.tile(cos64, (2, 1)))  # [128, S]
    sin2n = np.ascontiguousarray(np.tile(sin64s, (2, 1)))
    kk = np.arange(128)[:, None]
    cc = np.arange(896)[None, :]
    umask = (cc >= 384 + kk).astype(np.float32)  # [128, 896]
    return cos2, sin2n, umask


_NC_CACHE = None


def kernel(x, w_qkv, w_out):
    global _NC_CACHE
    x = np.asarray(x, dtype=np.float32)
    w_qkv = np.asarray(w_qkv, dtype=np.float32)
    w_out = np.asarray(w_out, dtype=np.float32)

    cos2, sin2n, umask = _host_tables()
    wq = w_qkv[:, 0:D]
    wk = w_qkv[:, D : 2 * D]
    wv = w_qkv[:, 2 * D : 3 * D]

    in_maps = []
    for c in range(8):
        b, hg = c // 4, c % 4
        cols = slice(hg * CV, (hg + 1) * CV)
        in_maps.append(
            {
                "xT": np.ascontiguousarray(x[b].T),
                "w_qk": np.ascontiguousarray(
                    np.concatenate([wq[:, cols], wk[:, cols]], axis=1)
                ),
                "w_v": np.ascontiguousarray(wv[:, cols]),
                "w_o": np.ascontiguousarray(w_out[cols, :]),
                "cos2": cos2,
                "sin2n": sin2n,
                "umask": umask,
                "ones4": np.ones((128, 4, 1), dtype=np.float32),
            }
        )

    if _NC_CACHE is None:
        _NC_CACHE = _build_nc()
    res = run_bass_kernel_spmd(_NC_CACHE, in_maps, core_ids=list(range(8)))
    out = np.zeros((B, S, D), dtype=np.float32)
    for c in range(8):
        out[c // 4] += res.results[c]["out"]
    return out


# revision 8
# speedup vs baseline: 1.0317x; 1.0317x over previous
"""Causal self-attention with RoPE on 8 Trainium2 NeuronCores.

Problem (hardcoded): x [2, 2048, 1024] f32, w_qkv [1024, 3072], w_out [1024, 1024],
16 heads x head_dim 64, RoPE base 10000, causal softmax, out = attn @ w_out.

Sharding: DP over batch (2) x TP over head-groups (4 heads/core) = 8 cores.
Each core computes QKV for its 4 heads, full causal attention, and a partial
output projection against its 256 rows of w_out. Host sums the 4 partials per
batch element.

Per-core device layout (all matmul operands fp32r = full-rate PE):
  xT   [1024, 2048]  x[b].T
  w_qk [1024, 512]   columns: [Q h0..h3 (4*64) | K h0..h3]
  w_v  [1024, 256]
  w_o  [256, 1024]   rows of w_out for this head group
  QKT  4 tiles [128, 2048] = Q^T/K^T in [channel, seq] layout (2 heads/tile)
  Vsb  16 tiles [128, 4, 65] = V[k-tile] per head + ones column (denominator)
  scores^T strips [k=128, q<=1024] in PSUM -> exp on ACT -> P^T (fp32r)
  out^T[qc] [65, 512] PSUM accumulators: rows 0..63 = head out, row 64 = denom
"""
import numpy as np

import concourse.bacc as bacc
import concourse.tile as tile
from concourse import mybir
from concourse.bass_utils import run_bass_kernel_spmd

F32 = mybir.dt.float32
F32R = mybir.dt.float32r
EXP = mybir.ActivationFunctionType.Exp

B, S, D = 2, 2048, 1024
H, HD = 16, 64
HPC = 4          # heads per core
CQK = 2 * HPC * HD   # 512 qk channels per core
CV = HPC * HD        # 256 v channels per core
NKT = S // 128       # 16 k-tiles
NSC = S // 512       # 4 seq chunks
SCALE = 1.0 / np.sqrt(HD)
ROPE_BASE = 10000.0


def _build_nc():
    nc = bacc.Bacc(None, target_bir_lowering=False, debug=False)

    xT = nc.declare_dram_parameter("xT", [D, S], F32, isOutput=False)
    w_qk = nc.declare_dram_parameter("w_qk", [D, CQK], F32, isOutput=False)
    w_v = nc.declare_dram_parameter("w_v", [D, CV], F32, isOutput=False)
    w_o = nc.declare_dram_parameter("w_o", [CV, D], F32, isOutput=False)
    cos2 = nc.declare_dram_parameter("cos2", [128, S], F32, isOutput=False)
    sin2n = nc.declare_dram_parameter("sin2n", [128, S], F32, isOutput=False)
    umask = nc.declare_dram_parameter("umask", [128, 896], F32, isOutput=False)
    ones4 = nc.declare_dram_parameter("ones4", [128, 4, 1], F32, isOutput=False)
    out = nc.declare_dram_parameter("out", [S, D], F32, isOutput=True)

    with tile.TileContext(nc) as tc:
        with (
            tc.tile_pool(name="const", bufs=1) as const,
            tc.tile_pool(name="qkt", bufs=1) as qkt_pool,
            tc.tile_pool(name="vsb", bufs=1) as vsb_pool,
            tc.tile_pool(name="rot", bufs=3) as rot_pool,
            tc.tile_pool(name="ps", bufs=1, space="PSUM") as ps,
        ):
            # ---- constants ----
            cos_sb = const.tile([128, S], F32R)
            sin_sb = const.tile([128, S], F32R)
            u_sb = const.tile([128, 896], F32R)
            nc.sync.dma_start(out=cos_sb, in_=cos2[:, :].bitcast(F32R))
            nc.sync.dma_start(out=sin_sb, in_=sin2n[:, :].bitcast(F32R))
            nc.sync.dma_start(out=u_sb, in_=umask[:, :].bitcast(F32R))
            ones_sb = const.tile([128, 4, 1], F32R)
            nc.sync.dma_start(out=ones_sb, in_=ones4[:, :, :].bitcast(F32R))
            wo_sb = [const.tile([128, D], F32R, name=f"wo{i}") for i in range(2)]
            for i in range(2):
                nc.sync.dma_start(
                    out=wo_sb[i], in_=w_o[i * 128 : (i + 1) * 128, :].bitcast(F32R)
                )

            # persistent outputs of phase 1
            QKT = [qkt_pool.tile([128, S], F32R, name=f"qkt{t}") for t in range(4)]
            Vsb = [vsb_pool.tile([128, HPC, 65], F32R, name=f"v{k}") for k in range(NKT)]

            # ---- phase 1: QKV projection ----
            with tc.tile_pool(name="p1", bufs=1) as p1:
                wqk_sb = [p1.tile([128, CQK], F32R, name=f"wqk{d}") for d in range(8)]
                wv_sb = [p1.tile([128, CV], F32R, name=f"wv{d}") for d in range(8)]
                for d in range(8):
                    nc.sync.dma_start(
                        out=wqk_sb[d],
                        in_=w_qk[d * 128 : (d + 1) * 128, :].bitcast(F32R),
                    )
                # xT tiles chunked [d-tile 128, s-chunk 512]; sc0 first so PE starts ASAP
                xt_sb = {}
                for sc in range(NSC):
                    for d in range(8):
                        t = p1.tile([128, 512], F32R, name=f"xt{sc}_{d}")
                        nc.sync.dma_start(
                            out=t,
                            in_=xT[
                                d * 128 : (d + 1) * 128, sc * 512 : (sc + 1) * 512
                            ].bitcast(F32R),
                        )
                        xt_sb[(sc, d)] = t
                    if sc == 0:
                        for d in range(8):
                            nc.sync.dma_start(
                                out=wv_sb[d],
                                in_=w_v[d * 128 : (d + 1) * 128, :].bitcast(F32R),
                            )

                def rope(t):
                    rot = rot_pool.tile([128, S], F32R, name="rope_rot")
                    for blk in range(4):
                        sp = (blk ^ 1) * 32  # 32<->0, 96<->64 swap per head
                        nc.gpsimd.dma_start(
                            out=rot[blk * 32 : blk * 32 + 32, :],
                            in_=QKT[t][sp : sp + 32, :],
                        )
                    nc.vector.tensor_mul(rot, rot, sin_sb)
                    nc.vector.tensor_mul(QKT[t], QKT[t], cos_sb)
                    nc.vector.tensor_add(QKT[t], QKT[t], rot)

                def qk_group(ct, sc):
                    qp = ps.tile([128, 1024], F32, tag="strip", bufs=2, name=f"qk{sc}_{ct}")
                    for d in range(8):
                        nc.tensor.matmul(
                            qp[:, 0:512],
                            wqk_sb[d][:, ct * 128 : (ct + 1) * 128],
                            xt_sb[(sc, d)],
                            start=(d == 0),
                            stop=(d == 7),
                        )
                    nc.vector.tensor_copy(
                        QKT[ct][:, sc * 512 : (sc + 1) * 512], qp[:, 0:512]
                    )

                def v_group(st):
                    sc, sti = st // 4, st % 4
                    vp = ps.tile([128, 4, 64], F32, tag="strip", bufs=2, name=f"vps{st}")
                    for d in range(8):
                        nc.tensor.matmul(
                            vp,
                            xt_sb[(sc, d)][:, sti * 128 : (sti + 1) * 128],
                            wv_sb[d],
                            start=(d == 0),
                            stop=(d == 7),
                        )
                    nc.vector.tensor_copy(Vsb[st][:, :, 0:64], vp)
                    nc.vector.tensor_copy(Vsb[st][:, :, 64:65], ones_sb)

                # Q/K for heads 0,1 first, each tile roped as soon as complete;
                # V interleaved so attention heads 0/1 can start while Q/K for
                # heads 2/3 (ct 1,3) still projects.
                for ct in (0, 2):
                    for sc in range(NSC):
                        qk_group(ct, sc)
                    rope(ct)
                for st in range(NKT):
                    v_group(st)
                for ct in (1, 3):
                    for sc in range(NSC):
                        qk_group(ct, sc)
                    rope(ct)

            # ---- phase 2: attention per head ----
            with (
                tc.tile_pool(name="pp", bufs=4) as pp,
                tc.tile_pool(name="attn", bufs=1) as attn_pool,
                tc.tile_pool(name="nrm", bufs=3) as nrm,
                tc.tile_pool(name="outp", bufs=4) as outp,
            ):
                attnT = [
                    [
                        attn_pool.tile([128, 512], F32R, name=f"attnT{qc}_{ct}")
                        for ct in range(2)
                    ]
                    for qc in range(NSC)
                ]
                for h in range(HPC):
                    qt = QKT[h // 2]
                    kt_t = QKT[2 + h // 2]
                    hh = h % 2
                    outT = [
                        ps.tile([128, 512], F32, tag="outT", bufs=4, name=f"outT{h}_{qc}")
                        for qc in range(NSC)
                    ]
                    for kt in range(NKT):
                        qc0 = kt // 4
                        c0 = qc0 * 512
                        width = S - c0
                        off = c0
                        first = True
                        while off < S:
                            w = min(1024, S - off)
                            sps = ps.tile(
                                [128, 1024], F32, tag="strip", bufs=2, name=f"s{h}_{kt}_{off}"
                            )
                            for j in range(w // 512):
                                nc.tensor.matmul(
                                    sps[:, j * 512 : (j + 1) * 512],
                                    kt_t[hh * 64 : hh * 64 + 64, kt * 128 : (kt + 1) * 128],
                                    qt[hh * 64 : hh * 64 + 64, off + j * 512 : off + (j + 1) * 512],
                                    start=True,
                                    stop=True,
                                )
                            p_t = pp.tile([128, 1024], F32R, name="p_t")
                            nc.scalar.activation(
                                p_t[:, 0:w], sps[:, 0:w], EXP, scale=SCALE
                            )
                            if first:
                                r = kt % 4
                                nc.vector.tensor_mul(
                                    p_t[:, 0:512],
                                    p_t[:, 0:512],
                                    u_sb[:, 384 - 128 * r : 896 - 128 * r],
                                )
                                first = False
                            for j in range(w // 512):
                                qc = (off + j * 512) // 512
                                nc.tensor.matmul(
                                    outT[qc][0:65, :],
                                    Vsb[kt][:, h, :],
                                    p_t[:, j * 512 : (j + 1) * 512],
                                    start=(kt == 0),
                                    stop=(kt == qc * 4 + 3),
                                )
                            off += w
                        # normalize finished q-chunk (kt = qc*4+3 just stopped)
                        if kt % 4 == 3:
                            qc = qc0
                            dn = nrm.tile([1, 512], F32, name="dn")
                            nc.vector.tensor_copy(dn, outT[qc][64:65, :])
                            bc = nrm.tile([64, 512], F32, name="bc")
                            nc.gpsimd.partition_broadcast(bc, dn)
                            rc = nrm.tile([64, 512], F32, name="rc")
                            nc.vector.reciprocal(rc, bc)
                            nc.vector.tensor_mul(
                                attnT[qc][h // 2][hh * 64 : hh * 64 + 64, :],
                                outT[qc][0:64, :],
                                rc,
                            )

                # ---- phase 3: output projection ----
                for st in range(16):
                    qc = st // 4
                    sl = (st % 4) * 128
                    for ec in range(2):
                        op = ps.tile(
                            [128, 1024], F32, tag="strip", bufs=2, name=f"op{st}_{ec}"
                        )
                        for ct in range(2):
                            nc.tensor.matmul(
                                op[:, 0:512],
                                attnT[qc][ct][:, sl : sl + 128],
                                wo_sb[ct][:, ec * 512 : (ec + 1) * 512],
                                start=(ct == 0),
                                stop=(ct == 1),
                            )
                        ob = outp.tile([128, 512], F32, name="ob")
                        nc.vector.tensor_copy(ob, op[:, 0:512])
                        nc.sync.dma_start(
                            out=out[st * 128 : (st + 1) * 128, ec * 512 : (ec + 1) * 512],
                            in_=ob,
                        )
    nc.compile()
    return nc


def _host_tables():
    half = HD // 2
    inv_freq = 1.0 / (ROPE_BASE ** (np.arange(0, half, dtype=np.float64) / half))
    ang = np.arange(S, dtype=np.float64)[:, None] * inv_freq[None, :]  # [S, 32]
    cosT = np.cos(ang).T.astype(np.float32)  # [32, S]
    sinT = np.sin(ang).T.astype(np.float32)
    cos64 = np.concatenate([cosT, cosT], axis=0)  # [64, S]
    sin64s = np.concatenate([-sinT, sinT], axis=0)  # sign-folded rotate_half
    cos2 = np.ascontiguousarray(np.tile(cos64, (2, 1)))  # [128, S]
    sin2n = np.ascontiguousarray(np.tile(sin64s, (2, 1)))
    kk = np.arange(128)[:, None]
    cc = np.arange(896)[None, :]
    umask = (cc >= 384 + kk).astype(np.float32)  # [128, 896]
    return cos2, sin2n, umask


_NC_CACHE = None


def kernel(x, w_qkv, w_out):
    global _NC_CACHE
    x = np.asarray(x, dtype=np.float32)
    w_qkv = np.asarray(w_qkv, dtype=np.float32)
    w_out = np.asarray(w_out, dtype=np.float32)

    cos2, sin2n, umask = _host_tables()
    wq = w_qkv[:, 0:D]
    wk = w_qkv[:, D : 2 * D]
    wv = w_qkv[:, 2 * D : 3 * D]

    in_maps = []
    for c in range(8):
        b, hg = c // 4, c % 4
        cols = slice(hg * CV, (hg + 1) * CV)
        in_maps.append(
            {
                "xT": np.ascontiguousarray(x[b].T),
                "w_qk": np.ascontiguousarray(
                    np.concatenate([wq[:, cols], wk[:, cols]], axis=1)
                ),
                "w_v": np.ascontiguousarray(wv[:, cols]),
                "w_o": np.ascontiguousarray(w_out[cols, :]),
                "cos2": cos2,
                "sin2n": sin2n,
                "umask": umask,
                "ones4": np.ones((128, 4, 1), dtype=np.float32),
            }
        )

    if _NC_CACHE is None:
        _NC_CACHE = _build_nc()
    res = run_bass_kernel_spmd(_NC_CACHE, in_maps, core_ids=list(range(8)))
    out = np.zeros((B, S, D), dtype=np.float32)
    for c in range(8):
        out[c // 4] += res.results[c]["out"]
    return out
